# revision 44
# baseline (speedup 1.0000x reference)
"""Trainium2 Bass kernel for nn_BGguidedLoss (BG-guided loss function).

Strategy: pure data-parallel over 8 NeuronCores. Each core processes
N/8 = 524288 rays laid out as [128 partitions x 4096 rays/partition].

v4 design notes (all w.r.t. the TimelineSim cost model):
  - fp16 intermediates: DVE tensor_tensor runs 2x (0.52 ns/elem) and
    tensor_scalar 4x (0.26 ns/elem) on packed 2-byte SBUF operands.
  - fp32 inputs are cast+deinterleaved to planar fp16 once per tile.
  - Activation-table discipline: u is loaded/cast up front, Ln+Exp run
    before the tile loop; the loop uses only Copy/Square (present in
    every table set); Sqrt+Sigmoid run once at the end -> 5 loads.
  - The idle PE (tensor) engine computes all difference tensors
    (gt-bg, gt-fg, dh, dv) as +/-identity matmuls accumulating in
    PSUM; ACT squares read PSUM directly.
  - GPSIMD (Pool) takes the comparisons, the hue divide and the
    sector assembly (scalar_tensor_tensor).
  - Per-ray loss folded as loss_i = ssqB/3 - mask*(ssqB/3 - FGp).
    FGp and t = ssqB/3 - FGp are computed inside the loop (shifted by
    SHIFT tiles); the tail is only sqrt+sigmoid+mask-mult+accumulate.
"""

import numpy as np

N_TOTAL = 4194304
N_CORES = 8
NC_RAYS = N_TOTAL // N_CORES          # 524288 rays per core
P = 128                               # partitions
FPP = NC_RAYS // P                    # 4096 rays per partition
K = 512                               # rays per partition per tile
NIT = FPP // K                        # tile iterations
NQ = 4                                # u prefetch quarters
KQ = FPP // NQ
SHIFT = 3                             # fgp tile shift
EPS = float(2.0 ** -13)               # divide guard (fp16-normal)
LN6 = float(np.log(np.float32(6.0)))

_CACHE = {}

# tunables (see sweep): tail/fgp/u/sum/dd structure modes
CFG = {
    "tail": "fpp",       # 'fpp' | 'halves'
    "fgp": "fpp",        # 'fpp' | 'inloop'
    "u": "tile",         # 'tile' | 'qdve' | 'qact' | 'qpool'
    "sum": "split",      # 'split' | 'fused'
    "dd": "fused",       # 'split' | 'fused'
    "shift": 3,
    "ffp": "dve",
}


def _build(full_variant: bool):
    import concourse.bacc as bacc
    import concourse.mybir as mybir
    import concourse.tile as tile

    f32 = mybir.dt.float32
    f16 = mybir.dt.float16
    op = mybir.AluOpType
    act = mybir.ActivationFunctionType

    nc = bacc.Bacc("TRN2", debug=False)

    # register EPS as a const AP so activation() accepts it as a bias
    _ct = nc.alloc_sbuf_tensor("const-f32-eps", [128, 1], f32)
    nc.gpsimd.memset(_ct.ap(), EPS)
    nc.const_aps.aps[(f32, EPS)] = _ct.ap()

    gt_d = nc.dram_tensor("gt_s", [NC_RAYS, 3], f32, kind="ExternalInput")
    bg_d = nc.dram_tensor("bg_s", [NC_RAYS, 3], f32, kind="ExternalInput")
    out_d = nc.dram_tensor("out_s", [P, 2], f32, kind="ExternalOutput")
    if full_variant:
        fg_d = nc.dram_tensor("fg_s", [NC_RAYS, 3], f32, kind="ExternalInput")
        u_d = nc.dram_tensor("u_s", [NC_RAYS], f32, kind="ExternalInput")
        prm_d = nc.dram_tensor("prm_s", [P, 2], f32, kind="ExternalInput")
        eye_d = nc.dram_tensor("eye_s", [P, P], f16, kind="ExternalInput")

    gt_v = gt_d.ap().rearrange("(p f) c -> p (f c)", p=P)
    bg_v = bg_d.ap().rearrange("(p f) c -> p (f c)", p=P)
    if full_variant:
        fg_v = fg_d.ap().rearrange("(p f) c -> p (f c)", p=P)
        u_v = u_d.ap().rearrange("(p f) -> p f", p=P)
    out_v = out_d.ap()

    with tile.TileContext(nc) as tc:
        with (
            tc.tile_pool(name="pin", bufs=2) as pin,
            tc.tile_pool(name="ptmp", bufs=1) as ptmp,
            tc.tile_pool(name="pers", bufs=1) as pers,
            tc.tile_pool(name="ppsum", bufs=1, space="PSUM") as ppsum,
        ):
            TTV = nc.vector.tensor_tensor
            TTP = nc.gpsimd.tensor_tensor
            STP = nc.gpsimd.scalar_tensor_tensor
            TSP = nc.vector.tensor_scalar
            ACT = nc.scalar.activation
            MM = nc.tensor.matmul

            if not full_variant:
                accT = pers.tile([P, 1], f32, tag="accT")
                nc.vector.memset(accT, 0.0)
                for t in range(NIT):
                    sl = slice(t * 3 * K, (t + 1) * 3 * K)
                    g = pin.tile([P, 3 * K], f32, tag="g", name=f"g{t}")
                    b = pin.tile([P, 3 * K], f32, tag="b", name=f"b{t}")
                    nc.sync.dma_start(g, gt_v[:, sl])
                    nc.sync.dma_start(b, bg_v[:, sl])
                    e = ptmp.tile([P, 3 * K], f32, tag="e", bufs=2,
                                  name=f"e{t}")
                    TTV(e, g, b, op.subtract)
                    ACT(e, e, act.Square)
                    acc_t = ptmp.tile([P, 1], f32, tag="acc_t", bufs=2,
                                      name=f"acc{t}")
                    TSP(e, e, 1.0, None, op.mult, op.add, accum_out=acc_t)
                    TTV(accT, accT, acc_t, op.add)
                acc2 = pers.tile([P, 2], f32, tag="acc2")
                nc.vector.memset(acc2, 0.0)
                nc.vector.tensor_copy(acc2[:, 0:1], accT)
                nc.sync.dma_start(out_v, acc2)
            else:
                prm = pers.tile([P, 2], f32, tag="prm")
                eye = pers.tile([P, P], f16, tag="eye")
                neye = pers.tile([P, P], f16, tag="neye")
                eye6 = pers.tile([P, P], f16, tag="eye6")
                ney6 = pers.tile([P, P], f16, tag="ney6")
                nc.sync.dma_start(prm, prm_d.ap())
                nc.sync.dma_start(eye, eye_d.ap())
                TSP(neye, eye, -1.0, 0.0, op.mult, op.add)
                TSP(eye6, eye, 1.0 / 6.0, 0.0, op.mult, op.add)
                TSP(ney6, eye, -1.0 / 6.0, 0.0, op.mult, op.add)

                # persistent per-ray fp16 arrays; ssqBF[:,0,:]=B, [:,1,:]=F
                ssqBF = pers.tile([P, 2 * FPP], f16, tag="ssqBF")
                ssqBFv = ssqBF.rearrange("p (i f) -> p i f", i=2)
                ssqB = ssqBFv[:, 0]
                ssqF = ssqBFv[:, 1]
                dsA = pers.tile([P, FPP], f16, tag="dsA")
                uA = pers.tile([P, FPP], f16, tag="uA")
                iu6 = pers.tile([P, FPP], f16, tag="iu6")

                lnu = uA
                fgp_done = []
                mt_done = []

                TILES = [(0, K // 2), (K // 2, K // 2)] + \
                    [(tt * K, K) for tt in range(1, NIT)]
                NTI = len(TILES)

                def do_fgp(t):
                    """t = ssqB/3 - FGp for tile t, written into ssqB."""
                    ofs, kk = TILES[t]
                    sl1 = slice(ofs, ofs + kk)
                    fgp = ptmp.tile([P, K], f16, tag="fgp", bufs=2,
                                    name=f"fgp{t}")[:, :kk]
                    TTV(fgp, ssqF[:, sl1], iu6[:, sl1], op.mult)
                    TTV(fgp, fgp, lnu[:, sl1], op.add)
                    tBt = ptmp.tile([P, K], f16, tag="tBt", bufs=2,
                                    name=f"tBt{t}")[:, :kk]
                    acB = ptmp.tile([P, 1], f32, tag="acB", bufs=NIT + 1,
                                    name=f"acB{t}")
                    TSP(tBt, ssqB[:, sl1], 1.0 / 3.0, None, op.mult,
                        op.add, accum_out=acB)
                    TTV(ssqB[:, sl1], tBt, fgp, op.subtract)
                    fgp_done.append(acB)

                for t, (ofs, kk) in enumerate(TILES):
                    sl3 = slice(ofs * 3, (ofs + kk) * 3)
                    sl1 = slice(ofs, ofs + kk)
                    gb = pin.tile([P, 6 * K], f32, tag="gb", name=f"gb{t}")
                    ff = pin.tile([P, 3 * K], f32, tag="ff", name=f"ff{t}")
                    if CFG["u"] in ("qdve", "qact") and t < NQ:
                        # u quarters interleaved with the first tiles
                        sq = slice(t * KQ, (t + 1) * KQ)
                        uq = pin.tile([P, KQ], f32, tag="uq",
                                      name=f"uq{t}")
                        nc.sync.dma_start(uq, u_v[:, sq])
                        if CFG["u"] == "qdve":
                            nc.vector.tensor_copy(uA[:, sq], uq)
                        else:
                            ACT(uA[:, sq], uq, act.Copy)
                    elif CFG["u"] == "tile":
                        uq = pin.tile([P, K], f32, tag="uq",
                                      name=f"uq{t}")
                        nc.sync.dma_start(uq[:, :kk], u_v[:, sl1])
                        ACT(uA[:, sl1], uq[:, :kk], act.Copy)
                    elif CFG["u"] == "qpool" and t < NQ:
                        sq = slice(t * KQ, (t + 1) * KQ)
                        uq = pin.tile([P, KQ], f32, tag="uq",
                                      name=f"uq{t}")
                        nc.sync.dma_start(uq, u_v[:, sq])
                        nc.gpsimd.tensor_copy(uA[:, sq], uq)
                    nc.sync.dma_start(gb[:, :3 * kk], gt_v[:, sl3])
                    nc.sync.dma_start(gb[:, 3 * kk:6 * kk], bg_v[:, sl3])
                    nc.sync.dma_start(ff[:, :3 * kk], fg_v[:, sl3])

                    # strided fp32 channel views
                    gb32 = gb[:, :6 * kk].rearrange("p (i k c) -> p c i k", i=2, c=3)
                    gb32f = gb[:, :6 * kk].rearrange("p (i k c) -> p c (i k)", i=2, c=3)

                    # planar fp16 cast: layout [P, (c, i, k)]
                    gbP = ptmp.tile([P, 6 * K], f16, tag="gbP", bufs=2,
                                    name=f"gbP{t}")
                    gbPv = gbP[:, :6 * kk].rearrange("p (c i k) -> p c i k", i=2, c=3)
                    ACT(gbPv, gb32, act.Copy)
                    ffP = ptmp.tile([P, 3 * K], f16, tag="ffP", bufs=2,
                                    name=f"ffP{t}")
                    ffPv = ffP[:, :3 * kk].rearrange("p (c k) -> p c k", c=3)
                    ff32 = ff[:, :3 * kk].rearrange("p (k c) -> p c k", c=3)
                    if CFG.get("ffp", "dve") == "pool":
                        nc.gpsimd.tensor_copy(ffPv, ff32)
                    else:
                        ACT(ffPv, ff32, act.Copy)

                    r = gbP[:, 0:2 * kk]
                    g = gbP[:, 2 * kk:4 * kk]
                    b = gbP[:, 4 * kk:6 * kk]

                    def t2k(nm, bufs=4):
                        tl = ptmp.tile([P, 2 * K], f16, tag=nm, bufs=bufs,
                                       name=f"{nm}{t}")
                        return tl[:, :2 * kk]

                    # ---- hue chain (both images batched on [P, 2K]) ----
                    # GPSIMD ISA only supports add/sub/mult tensor ops,
                    # and DVE has no divide: reciprocal_approx_fast (fp32)
                    # replaces num/d. Pool takes off-critical mults.
                    Px = t2k("t0"); TTV(Px, g, b, op.max)
                    Py = t2k("t1"); TTV(Py, g, b, op.min)
                    c1 = t2k("c1", 2)
                    TTV(c1, g, b, op.is_lt)                      # g < b
                    M = t2k("Mt", 3); TTV(M, r, Px, op.max)
                    Qw = t2k("t2"); TTV(Qw, r, Px, op.min)
                    c2 = t2k("c2", 3)
                    TTV(c2, r, Px, op.is_lt)                     # r < Px
                    m = t2k("t3"); TTV(m, r, Py, op.min)
                    d = t2k("t4"); TTV(d, M, m, op.subtract)
                    d32 = ptmp.tile([P, 2 * K], f32, tag="d32", bufs=2,
                                    name=f"d32{t}")
                    # cast + eps-guard in one ACT pass (d+EPS > 0)
                    ACT(d32[:, :2 * kk], d, act.Identity, bias=EPS)
                    rc = ptmp.tile([P, 2 * K], f32, tag="rc", bufs=2,
                                   name=f"rc{t}")
                    nc.vector.reciprocal_approx_fast(rc[:, :2 * kk], d32[:, :2 * kk])
                    num = t2k("t5"); TTV(num, Qw, Py, op.subtract)
                    x = t2k("t3"); TTV(x, c1, c2, op.not_equal)
                    s = t2k("t1"); TSP(s, x, -2.0, 1.0, op.mult, op.add)
                    snum = t2k("t0"); TTV(snum, s, num, op.mult)
                    sn = t2k("t2")
                    TTV(sn, snum, rc[:, :2 * kk], op.mult)
                    w = t2k("t3"); TTP(w, c1, c2, op.mult)       # c1*c2
                    zc = t2k("t5"); TTV(zc, c2, w, op.add)
                    TSP(zc, zc, 2.0, 0.0, op.mult, op.add)       # 2*zc
                    H = t2k("t4"); TTV(H, zc, sn, op.add)        # 2*zc+sn
                    t6 = t2k("t3"); TSP(t6, H, 3.0, 6.0, op.is_ge, op.mult)

                    # ---- PE: difference tensors into PSUM ----
                    # pB/pF first: they only need the casts; pd (which
                    # waits for the end of the hue chain) goes last so it
                    # does not block PE's in-order queue.
                    gtPv = gbPv[:, :, 0, :]                      # [P, 3, K]
                    bgPv = gbPv[:, :, 1, :]
                    pB = ppsum.tile([P, 3 * K], f32, tag="pB")
                    pBv = pB[:, :3 * kk].rearrange("p (c k) -> p c k", c=3)
                    pF = ppsum.tile([P, 3 * K], f32, tag="pF")
                    pFv = pF[:, :3 * kk].rearrange("p (c k) -> p c k", c=3)
                    for c in range(3):
                        MM(pBv[:, c], eye, gtPv[:, c], start=True, stop=False)
                        MM(pBv[:, c], neye, bgPv[:, c], start=False, stop=True)
                        MM(pFv[:, c], eye, gtPv[:, c], start=True, stop=False)
                        MM(pFv[:, c], neye, ffPv[:, c], start=False, stop=True)

                    # pd[:, :K] = dh*(1/6) via eye6, pd[:, K:] = dv
                    # dh = (H - t6)[:, :K] - (H - t6)[:, K:]
                    pd = ppsum.tile([P, 2 * K], f32, tag="pd")
                    ph = pd[:, :kk]
                    pv = pd[:, kk:2 * kk]
                    MM(pv, eye, M[:, :kk], start=True, stop=False)
                    MM(pv, neye, M[:, kk:], start=False, stop=True)
                    MM(ph, eye6, H[:, :kk], start=True, stop=False)
                    MM(ph, ney6, H[:, kk:], start=False, stop=False)
                    MM(ph, ney6, t6[:, :kk], start=False, stop=False)
                    MM(ph, eye6, t6[:, kk:], start=False, stop=True)

                    # ---- ACT squares (PSUM -> SBUF fp16) ----
                    if CFG["sum"] == "fused":
                        # eBF2[:, :3K] = eB^2, [:, 3K:] = eF^2
                        eBF2 = ptmp.tile([P, 6 * K], f16, tag="eBF2",
                                         bufs=2, name=f"eBF2{t}")
                        ACT(eBF2[:, :3 * K], pB, act.Square)
                        ACT(eBF2[:, 3 * K:], pF, act.Square)
                        ev = eBF2.rearrange("p (i c k) -> p c i k",
                                            i=2, c=3)
                        q2 = ptmp.tile([P, 2 * K], f16, tag="q2t",
                                       bufs=2, name=f"q2t{t}")
                        q2v = q2.rearrange("p (i k) -> p i k", i=2)
                        ssq_out = ssqBFv[:, :, sl1]
                        TTV(q2v, ev[:, 0], ev[:, 1], op.add)
                        TTV(ssq_out, q2v, ev[:, 2], op.add)
                    else:
                        eB2 = ptmp.tile([P, 3 * K], f16, tag="eB2",
                                        bufs=2, name=f"eB2{t}")
                        eF2 = ptmp.tile([P, 3 * K], f16, tag="eF2",
                                        bufs=2, name=f"eF2{t}")
                        ACT(eB2[:, :3 * kk], pB[:, :3 * kk], act.Square)
                        ACT(eF2[:, :3 * kk], pF[:, :3 * kk], act.Square)
                        q = ptmp.tile([P, K], f16, tag="qt", bufs=2,
                                      name=f"qt{t}")[:, :kk]
                        TTV(q, eB2[:, :kk], eB2[:, kk:2 * kk], op.add)
                        TTV(ssqB[:, sl1], q, eB2[:, 2 * kk:3 * kk], op.add)
                        TTP(q, eF2[:, :kk], eF2[:, kk:2 * kk], op.add)
                        TTP(ssqF[:, sl1], q, eF2[:, 2 * kk:3 * kk], op.add)

                    if CFG["dd"] == "fused":
                        dd2 = ptmp.tile([P, 2 * K], f16, tag="dd2",
                                        bufs=2, name=f"dd2{t}")
                        ACT(dd2[:, :2 * kk], pd[:, :2 * kk], act.Square)
                        TTP(dsA[:, sl1], dd2[:, :kk], dd2[:, kk:2 * kk],
                            op.add)
                    else:
                        dh2 = ptmp.tile([P, K], f16, tag="dh2", bufs=2,
                                        name=f"dh2{t}")
                        dv2 = ptmp.tile([P, K], f16, tag="dv2", bufs=2,
                                        name=f"dv2{t}")
                        ACT(dh2, pd[:, :K], act.Square)
                        ACT(dv2, pd[:, K:], act.Square)
                        TTV(dsA[:, sl1], dh2, dv2, op.add)

                    if CFG["u"] in ("qdve", "qact", "qpool") and t == NQ:
                        # all u quarters cast; one table switch each
                        ACT(lnu, uA, act.Ln)
                        ACT(iu6, lnu, act.Exp, bias=prm[:, 1:2],
                            scale=-2.0)
                    elif CFG["u"] == "tile" and t == NTI - 1:
                        ACT(lnu, uA, act.Ln)
                        ACT(iu6, lnu, act.Exp, bias=prm[:, 1:2],
                            scale=-2.0)
                    SH = CFG["shift"]
                    if CFG["fgp"] == "inloop":
                        if t >= SH:
                            do_fgp(t - SH)
                        if t == NIT - 1:
                            for tt in range(NIT - SH, NIT):
                                do_fgp(tt)
                    if CFG["tail"] == "halves" and \
                            (t == NIT - 3 or t == NIT - 1):
                        # mask + mask*t for the finished half
                        hh = 0 if t == NIT - 3 else 1
                        HF = FPP // 2
                        sh = slice(hh * HF, (hh + 1) * HF)
                        ACT(dsA[:, sh], dsA[:, sh], act.Sqrt)
                        ACT(dsA[:, sh], dsA[:, sh], act.Sigmoid,
                            bias=prm[:, 0:1], scale=10.0)
                        if CFG["fgp"] == "inloop":
                            TTV(ssqB[:, sh], dsA[:, sh], ssqB[:, sh],
                                op.mult)
                            acM = ptmp.tile([P, 1], f32, tag="acM",
                                            bufs=2, name=f"acM{hh}")
                            TSP(iu6[:, sh], ssqB[:, sh], 1.0, None,
                                op.mult, op.add, accum_out=acM)
                            mt_done.append(acM)

                # ---- tail ----
                accB = pers.tile([P, 1], f32, tag="accB")
                accMT = pers.tile([P, 1], f32, tag="accMT")
                if CFG["fgp"] == "fpp":
                    for tt in range(NTI):
                        do_fgp(tt)
                if CFG["tail"] == "fpp":
                    ACT(dsA, dsA, act.Sqrt)
                    ACT(dsA, dsA, act.Sigmoid, bias=prm[:, 0:1],
                        scale=10.0)
                    TTV(ssqB, dsA, ssqB, op.mult)
                    acM = ptmp.tile([P, 1], f32, tag="acM", bufs=2,
                                    name="acM0")
                    TSP(iu6, ssqB, 1.0, None, op.mult, op.add,
                        accum_out=acM)
                    mt_done.append(acM)
                elif CFG["fgp"] == "fpp":
                    # halves tail but fgp ran post-loop: mask-mult now
                    for hh in range(2):
                        HF = FPP // 2
                        sh = slice(hh * HF, (hh + 1) * HF)
                        TTV(ssqB[:, sh], dsA[:, sh], ssqB[:, sh],
                            op.mult)
                        acM = ptmp.tile([P, 1], f32, tag="acM", bufs=2,
                                        name=f"acM{hh}")
                        TSP(iu6[:, sh], ssqB[:, sh], 1.0, None, op.mult,
                            op.add, accum_out=acM)
                        mt_done.append(acM)
                nc.vector.tensor_copy(accB, fgp_done[0])
                for acB in fgp_done[1:]:
                    TTV(accB, accB, acB, op.add)
                nc.vector.tensor_copy(accMT, mt_done[0])
                for acM in mt_done[1:]:
                    TTV(accMT, accMT, acM, op.add)
                accO = pers.tile([P, 2], f32, tag="accO")
                nc.vector.tensor_copy(accO[:, 0:1], accB)
                nc.vector.tensor_copy(accO[:, 1:2], accMT)
                nc.sync.dma_start(out_v, accO)

    nc.compile()
    return nc


def _get_nc(full_variant: bool):
    key = (bool(full_variant), tuple(sorted(CFG.items())))
    if key not in _CACHE:
        _CACHE[key] = _build(full_variant)
    return _CACHE[key]


def _run(inputs, trace=False):
    from concourse.bass_utils import run_bass_kernel_spmd

    gt = np.ascontiguousarray(np.asarray(inputs["gt"], dtype=np.float32))
    bg = np.ascontiguousarray(np.asarray(inputs["BG_map"], dtype=np.float32))
    it = int(np.asarray(inputs["iter"]))
    full = it > 300

    if full:
        fg = np.ascontiguousarray(np.asarray(inputs["FG_map"],
                                             dtype=np.float32))
        u = np.ascontiguousarray(
            np.asarray(inputs["FG_uncertainties"], dtype=np.float32)
        ).reshape(-1)
        tp = float(np.asarray(inputs["threshold_param"]))
        thr = 1.414 * (1.0 - 1.0 / (1.0 + np.exp(-tp)))
        prm = np.zeros((P, 2), dtype=np.float32)
        prm[:, 0] = np.float32(-10.0 * thr)
        prm[:, 1] = np.float32(-LN6)
        eye = np.eye(P, dtype=np.float16)

    nc = _get_nc(full)
    in_maps = []
    for c in range(N_CORES):
        sl = slice(c * NC_RAYS, (c + 1) * NC_RAYS)
        m = {"gt_s": gt[sl], "bg_s": bg[sl]}
        if full:
            m["fg_s"] = fg[sl]
            m["u_s"] = u[sl]
            m["prm_s"] = prm
            m["eye_s"] = eye
        in_maps.append(m)

    res = run_bass_kernel_spmd(nc, in_maps, core_ids=list(range(N_CORES)),
                               trace=trace)
    parts = np.stack([r["out_s"] for r in res.results])  # [8, 128, 2]
    tot = parts.astype(np.float64).sum(axis=(0, 1))      # [2]
    if full:
        val = (tot[0] - tot[1]) / N_TOTAL
    else:
        val = tot[0] / (N_TOTAL * 3)
    return np.float32(val), res


def kernel(**inputs) -> np.ndarray:
    val, _ = _run(inputs, trace=False)
    return np.asarray(val, dtype=np.float32)


# revision 50
# speedup vs baseline: 1.0146x; 1.0146x over previous
"""Trainium2 Bass kernel for nn_BGguidedLoss (BG-guided loss function).

Strategy: pure data-parallel over 8 NeuronCores. Each core processes
N/8 = 524288 rays laid out as [128 partitions x 4096 rays/partition].

v4 design notes (all w.r.t. the TimelineSim cost model):
  - fp16 intermediates: DVE tensor_tensor runs 2x (0.52 ns/elem) and
    tensor_scalar 4x (0.26 ns/elem) on packed 2-byte SBUF operands.
  - fp32 inputs are cast+deinterleaved to planar fp16 once per tile.
  - Activation-table discipline: u is loaded/cast up front, Ln+Exp run
    before the tile loop; the loop uses only Copy/Square (present in
    every table set); Sqrt+Sigmoid run once at the end -> 5 loads.
  - The idle PE (tensor) engine computes all difference tensors
    (gt-bg, gt-fg, dh, dv) as +/-identity matmuls accumulating in
    PSUM; ACT squares read PSUM directly.
  - GPSIMD (Pool) takes the comparisons, the hue divide and the
    sector assembly (scalar_tensor_tensor).
  - Per-ray loss folded as loss_i = ssqB/3 - mask*(ssqB/3 - FGp).
    FGp and t = ssqB/3 - FGp are computed inside the loop (shifted by
    SHIFT tiles); the tail is only sqrt+sigmoid+mask-mult+accumulate.
"""

import numpy as np

N_TOTAL = 4194304
N_CORES = 8
NC_RAYS = N_TOTAL // N_CORES          # 524288 rays per core
P = 128                               # partitions
FPP = NC_RAYS // P                    # 4096 rays per partition
K = 512                               # rays per partition per tile
NIT = FPP // K                        # tile iterations
NQ = 4                                # u prefetch quarters
KQ = FPP // NQ
SHIFT = 3                             # fgp tile shift
EPS = float(2.0 ** -13)               # divide guard (fp16-normal)
LN6 = float(np.log(np.float32(6.0)))

_CACHE = {}

# tunables (see sweep): tail/fgp/u/sum/dd structure modes
CFG = {
    "tail": "fpp",       # 'fpp' | 'halves'
    "fgp": "fpp",        # 'fpp' | 'inloop'
    "u": "tile",         # 'tile' | 'qdve' | 'qact' | 'qpool'
    "sum": "split",      # 'split' | 'fused'
    "dd": "fused",       # 'split' | 'fused'
    "shift": 3,
    "ffp": "dve",
}


def _build(full_variant: bool):
    import concourse.bacc as bacc
    import concourse.mybir as mybir
    import concourse.tile as tile

    f32 = mybir.dt.float32
    f16 = mybir.dt.float16
    op = mybir.AluOpType
    act = mybir.ActivationFunctionType

    nc = bacc.Bacc("TRN2", debug=False)

    # register EPS as a const AP so activation() accepts it as a bias
    _ct = nc.alloc_sbuf_tensor("const-f32-eps", [128, 1], f32)
    nc.gpsimd.memset(_ct.ap(), EPS)
    nc.const_aps.aps[(f32, EPS)] = _ct.ap()

    gt_d = nc.dram_tensor("gt_s", [NC_RAYS, 3], f32, kind="ExternalInput")
    bg_d = nc.dram_tensor("bg_s", [NC_RAYS, 3], f32, kind="ExternalInput")
    out_d = nc.dram_tensor("out_s", [P, 2], f32, kind="ExternalOutput")
    if full_variant:
        fg_d = nc.dram_tensor("fg_s", [NC_RAYS, 3], f32, kind="ExternalInput")
        u_d = nc.dram_tensor("u_s", [NC_RAYS], f32, kind="ExternalInput")
        prm_d = nc.dram_tensor("prm_s", [P, 2], f32, kind="ExternalInput")
        eye_d = nc.dram_tensor("eye_s", [P, P], f16, kind="ExternalInput")

    gt_v = gt_d.ap().rearrange("(p f) c -> p (f c)", p=P)
    bg_v = bg_d.ap().rearrange("(p f) c -> p (f c)", p=P)
    if full_variant:
        fg_v = fg_d.ap().rearrange("(p f) c -> p (f c)", p=P)
        u_v = u_d.ap().rearrange("(p f) -> p f", p=P)
    out_v = out_d.ap()

    with tile.TileContext(nc) as tc:
        with (
            tc.tile_pool(name="pin", bufs=2) as pin,
            tc.tile_pool(name="ptmp", bufs=1) as ptmp,
            tc.tile_pool(name="pers", bufs=1) as pers,
            tc.tile_pool(name="ppsum", bufs=1, space="PSUM") as ppsum,
        ):
            TTV = nc.vector.tensor_tensor
            TTP = nc.gpsimd.tensor_tensor
            STP = nc.gpsimd.scalar_tensor_tensor
            TSP = nc.vector.tensor_scalar
            ACT = nc.scalar.activation
            MM = nc.tensor.matmul

            if not full_variant:
                accT = pers.tile([P, 1], f32, tag="accT")
                nc.vector.memset(accT, 0.0)
                for t in range(NIT):
                    sl = slice(t * 3 * K, (t + 1) * 3 * K)
                    g = pin.tile([P, 3 * K], f32, tag="g", name=f"g{t}")
                    b = pin.tile([P, 3 * K], f32, tag="b", name=f"b{t}")
                    nc.sync.dma_start(g, gt_v[:, sl])
                    nc.sync.dma_start(b, bg_v[:, sl])
                    e = ptmp.tile([P, 3 * K], f32, tag="e", bufs=2,
                                  name=f"e{t}")
                    TTV(e, g, b, op.subtract)
                    ACT(e, e, act.Square)
                    acc_t = ptmp.tile([P, 1], f32, tag="acc_t", bufs=2,
                                      name=f"acc{t}")
                    TSP(e, e, 1.0, None, op.mult, op.add, accum_out=acc_t)
                    TTV(accT, accT, acc_t, op.add)
                acc2 = pers.tile([P, 2], f32, tag="acc2")
                nc.vector.memset(acc2, 0.0)
                nc.vector.tensor_copy(acc2[:, 0:1], accT)
                nc.sync.dma_start(out_v, acc2)
            else:
                prm = pers.tile([P, 2], f32, tag="prm")
                eye = pers.tile([P, P], f16, tag="eye")
                neye = pers.tile([P, P], f16, tag="neye")
                eye6 = pers.tile([P, P], f16, tag="eye6")
                ney6 = pers.tile([P, P], f16, tag="ney6")
                nc.sync.dma_start(prm, prm_d.ap())
                nc.sync.dma_start(eye, eye_d.ap())
                TSP(neye, eye, -1.0, 0.0, op.mult, op.add)
                TSP(eye6, eye, 1.0 / 6.0, 0.0, op.mult, op.add)
                TSP(ney6, eye, -1.0 / 6.0, 0.0, op.mult, op.add)

                # persistent per-ray fp16 arrays; ssqBF[:,0,:]=B, [:,1,:]=F
                ssqBF = pers.tile([P, 2 * FPP], f16, tag="ssqBF")
                ssqBFv = ssqBF.rearrange("p (i f) -> p i f", i=2)
                ssqB = ssqBFv[:, 0]
                ssqF = ssqBFv[:, 1]
                dsA = pers.tile([P, FPP], f16, tag="dsA")
                uA = pers.tile([P, FPP], f16, tag="uA")
                iu6 = pers.tile([P, FPP], f16, tag="iu6")

                lnu = uA
                fgp_done = []
                mt_done = []

                TILES = [(0, K // 2), (K // 2, K // 2)] + \
                    [(tt * K, K) for tt in range(1, NIT)]
                NTI = len(TILES)

                def do_fgp(t):
                    """t = ssqB/3 - FGp for tile t, written into ssqB."""
                    ofs, kk = TILES[t]
                    sl1 = slice(ofs, ofs + kk)
                    fgp = ptmp.tile([P, K], f16, tag="fgp", bufs=2,
                                    name=f"fgp{t}")[:, :kk]
                    TTV(fgp, ssqF[:, sl1], iu6[:, sl1], op.mult)
                    TTV(fgp, fgp, lnu[:, sl1], op.add)
                    tBt = ptmp.tile([P, K], f16, tag="tBt", bufs=2,
                                    name=f"tBt{t}")[:, :kk]
                    acB = ptmp.tile([P, 1], f32, tag="acB", bufs=NIT + 1,
                                    name=f"acB{t}")
                    TSP(tBt, ssqB[:, sl1], 1.0 / 3.0, None, op.mult,
                        op.add, accum_out=acB)
                    TTV(ssqB[:, sl1], tBt, fgp, op.subtract)
                    fgp_done.append(acB)

                for t, (ofs, kk) in enumerate(TILES):
                    sl3 = slice(ofs * 3, (ofs + kk) * 3)
                    sl1 = slice(ofs, ofs + kk)
                    gb = pin.tile([P, 6 * K], f32, tag="gb", name=f"gb{t}")
                    ff = pin.tile([P, 3 * K], f32, tag="ff", name=f"ff{t}")
                    if CFG["u"] in ("qdve", "qact") and t < NQ:
                        # u quarters interleaved with the first tiles
                        sq = slice(t * KQ, (t + 1) * KQ)
                        uq = pin.tile([P, KQ], f32, tag="uq",
                                      name=f"uq{t}")
                        nc.sync.dma_start(uq, u_v[:, sq])
                        if CFG["u"] == "qdve":
                            nc.vector.tensor_copy(uA[:, sq], uq)
                        else:
                            ACT(uA[:, sq], uq, act.Copy)
                    elif CFG["u"] == "tile":
                        uq = pin.tile([P, K], f32, tag="uq",
                                      name=f"uq{t}")
                        nc.sync.dma_start(uq[:, :kk], u_v[:, sl1])
                        ACT(uA[:, sl1], uq[:, :kk], act.Copy)
                    elif CFG["u"] == "qpool" and t < NQ:
                        sq = slice(t * KQ, (t + 1) * KQ)
                        uq = pin.tile([P, KQ], f32, tag="uq",
                                      name=f"uq{t}")
                        nc.sync.dma_start(uq, u_v[:, sq])
                        nc.gpsimd.tensor_copy(uA[:, sq], uq)
                    nc.sync.dma_start(gb[:, :3 * kk], gt_v[:, sl3])
                    nc.sync.dma_start(gb[:, 3 * kk:6 * kk], bg_v[:, sl3])
                    nc.sync.dma_start(ff[:, :3 * kk], fg_v[:, sl3])

                    # strided fp32 channel views
                    gb32 = gb[:, :6 * kk].rearrange("p (i k c) -> p c i k", i=2, c=3)
                    gb32f = gb[:, :6 * kk].rearrange("p (i k c) -> p c (i k)", i=2, c=3)

                    # planar fp16 cast: layout [P, (c, i, k)]
                    gbP = ptmp.tile([P, 6 * K], f16, tag="gbP", bufs=2,
                                    name=f"gbP{t}")
                    gbPv = gbP[:, :6 * kk].rearrange("p (c i k) -> p c i k", i=2, c=3)
                    ACT(gbPv, gb32, act.Copy)
                    ffP = ptmp.tile([P, 3 * K], f16, tag="ffP", bufs=2,
                                    name=f"ffP{t}")
                    ffPv = ffP[:, :3 * kk].rearrange("p (c k) -> p c k", c=3)
                    ff32 = ff[:, :3 * kk].rearrange("p (k c) -> p c k", c=3)
                    if CFG.get("ffp", "dve") == "pool":
                        nc.gpsimd.tensor_copy(ffPv, ff32)
                    else:
                        ACT(ffPv, ff32, act.Copy)

                    r = gbP[:, 0:2 * kk]
                    g = gbP[:, 2 * kk:4 * kk]
                    b = gbP[:, 4 * kk:6 * kk]

                    def t2k(nm, bufs=4):
                        tl = ptmp.tile([P, 2 * K], f16, tag=nm, bufs=bufs,
                                       name=f"{nm}{t}")
                        return tl[:, :2 * kk]

                    # ---- hue chain (both images batched on [P, 2K]) ----
                    # GPSIMD ISA only supports add/sub/mult tensor ops,
                    # and DVE has no divide: reciprocal_approx_fast (fp32)
                    # replaces num/d. Pool takes off-critical mults.
                    Px = t2k("t0"); TTV(Px, g, b, op.max)
                    Py = t2k("t1"); TTV(Py, g, b, op.min)
                    c1 = t2k("c1", 2)
                    TTV(c1, g, b, op.is_lt)                      # g < b
                    M = t2k("Mt", 3); TTV(M, r, Px, op.max)
                    Qw = t2k("t2"); TTV(Qw, r, Px, op.min)
                    c2 = t2k("c2", 3)
                    TTV(c2, r, Px, op.is_lt)                     # r < Px
                    m = t2k("t3"); TTV(m, r, Py, op.min)
                    d = t2k("t4"); TTV(d, M, m, op.subtract)
                    d32 = ptmp.tile([P, 2 * K], f32, tag="d32", bufs=2,
                                    name=f"d32{t}")
                    # cast + eps-guard in one ACT pass (d+EPS > 0)
                    ACT(d32[:, :2 * kk], d, act.Identity, bias=EPS)
                    rc = ptmp.tile([P, 2 * K], f32, tag="rc", bufs=2,
                                   name=f"rc{t}")
                    nc.vector.reciprocal_approx_fast(rc[:, :2 * kk], d32[:, :2 * kk])
                    num = t2k("t5"); TTV(num, Qw, Py, op.subtract)
                    x = t2k("t3"); TTV(x, c1, c2, op.not_equal)
                    s = t2k("t1"); TSP(s, x, -2.0, 1.0, op.mult, op.add)
                    snum = t2k("t0"); TTV(snum, s, num, op.mult)
                    sn = t2k("t2")
                    TTV(sn, snum, rc[:, :2 * kk], op.mult)
                    w = t2k("t3"); TTP(w, c1, c2, op.mult)       # c1*c2
                    zc = t2k("t5"); TTV(zc, c2, w, op.add)
                    TSP(zc, zc, 2.0, 0.0, op.mult, op.add)       # 2*zc
                    H = t2k("t4"); TTV(H, zc, sn, op.add)        # 2*zc+sn
                    t6 = t2k("t3"); TSP(t6, H, 3.0, 6.0, op.is_ge, op.mult)

                    # ---- PE: difference tensors into PSUM ----
                    # pB/pF first: they only need the casts; pd (which
                    # waits for the end of the hue chain) goes last so it
                    # does not block PE's in-order queue.
                    gtPv = gbPv[:, :, 0, :]                      # [P, 3, K]
                    bgPv = gbPv[:, :, 1, :]
                    pB = ppsum.tile([P, 3 * K], f32, tag="pB")
                    pBv = pB[:, :3 * kk].rearrange("p (c k) -> p c k", c=3)
                    pF = ppsum.tile([P, 3 * K], f32, tag="pF")
                    pFv = pF[:, :3 * kk].rearrange("p (c k) -> p c k", c=3)
                    for c in range(3):
                        MM(pBv[:, c], eye, gtPv[:, c], start=True, stop=False)
                        MM(pBv[:, c], neye, bgPv[:, c], start=False, stop=True)
                        MM(pFv[:, c], eye, gtPv[:, c], start=True, stop=False)
                        MM(pFv[:, c], neye, ffPv[:, c], start=False, stop=True)

                    # pd[:, :K] = dh*(1/6) via eye6, pd[:, K:] = dv
                    # dh = (H - t6)[:, :K] - (H - t6)[:, K:]
                    pd = ppsum.tile([P, 2 * K], f32, tag="pd")
                    ph = pd[:, :kk]
                    pv = pd[:, kk:2 * kk]
                    MM(pv, eye, M[:, :kk], start=True, stop=False)
                    MM(pv, neye, M[:, kk:], start=False, stop=True)
                    MM(ph, eye6, H[:, :kk], start=True, stop=False)
                    MM(ph, ney6, H[:, kk:], start=False, stop=False)
                    MM(ph, ney6, t6[:, :kk], start=False, stop=False)
                    MM(ph, eye6, t6[:, kk:], start=False, stop=True)

                    # ---- ACT squares (PSUM -> SBUF fp16) ----
                    if CFG["sum"] == "fused":
                        # eBF2[:, :3K] = eB^2, [:, 3K:] = eF^2
                        eBF2 = ptmp.tile([P, 6 * K], f16, tag="eBF2",
                                         bufs=2, name=f"eBF2{t}")
                        ACT(eBF2[:, :3 * K], pB, act.Square)
                        ACT(eBF2[:, 3 * K:], pF, act.Square)
                        ev = eBF2.rearrange("p (i c k) -> p c i k",
                                            i=2, c=3)
                        q2 = ptmp.tile([P, 2 * K], f16, tag="q2t",
                                       bufs=2, name=f"q2t{t}")
                        q2v = q2.rearrange("p (i k) -> p i k", i=2)
                        ssq_out = ssqBFv[:, :, sl1]
                        TTV(q2v, ev[:, 0], ev[:, 1], op.add)
                        TTV(ssq_out, q2v, ev[:, 2], op.add)
                    else:
                        eB2 = ptmp.tile([P, 3 * K], f16, tag="eB2",
                                        bufs=2, name=f"eB2{t}")
                        eF2 = ptmp.tile([P, 3 * K], f16, tag="eF2",
                                        bufs=2, name=f"eF2{t}")
                        ACT(eB2[:, :3 * kk], pB[:, :3 * kk], act.Square)
                        ACT(eF2[:, :3 * kk], pF[:, :3 * kk], act.Square)
                        q = ptmp.tile([P, K], f16, tag="qt", bufs=2,
                                      name=f"qt{t}")[:, :kk]
                        TTV(q, eB2[:, :kk], eB2[:, kk:2 * kk], op.add)
                        TTV(ssqB[:, sl1], q, eB2[:, 2 * kk:3 * kk], op.add)
                        TTP(q, eF2[:, :kk], eF2[:, kk:2 * kk], op.add)
                        TTP(ssqF[:, sl1], q, eF2[:, 2 * kk:3 * kk], op.add)

                    if CFG["dd"] == "fused":
                        dd2 = ptmp.tile([P, 2 * K], f16, tag="dd2",
                                        bufs=2, name=f"dd2{t}")
                        ACT(dd2[:, :2 * kk], pd[:, :2 * kk], act.Square)
                        TTP(dsA[:, sl1], dd2[:, :kk], dd2[:, kk:2 * kk],
                            op.add)
                    else:
                        dh2 = ptmp.tile([P, K], f16, tag="dh2", bufs=2,
                                        name=f"dh2{t}")
                        dv2 = ptmp.tile([P, K], f16, tag="dv2", bufs=2,
                                        name=f"dv2{t}")
                        ACT(dh2, pd[:, :K], act.Square)
                        ACT(dv2, pd[:, K:], act.Square)
                        TTV(dsA[:, sl1], dh2, dv2, op.add)

                    if CFG["u"] in ("qdve", "qact", "qpool") and t == NQ:
                        # all u quarters cast; one table switch each
                        ACT(lnu, uA, act.Ln)
                        ACT(iu6, lnu, act.Exp, bias=prm[:, 1:2],
                            scale=-2.0)
                    elif CFG["u"] == "tile" and t == NTI - 1:
                        ACT(lnu, uA, act.Ln)
                        ACT(iu6, lnu, act.Exp, bias=prm[:, 1:2],
                            scale=-2.0)
                    SH = CFG["shift"]
                    if CFG["fgp"] == "inloop":
                        if t >= SH:
                            do_fgp(t - SH)
                        if t == NIT - 1:
                            for tt in range(NIT - SH, NIT):
                                do_fgp(tt)
                    if CFG["tail"] == "halves" and \
                            (t == NIT - 3 or t == NIT - 1):
                        # mask + mask*t for the finished half
                        hh = 0 if t == NIT - 3 else 1
                        HF = FPP // 2
                        sh = slice(hh * HF, (hh + 1) * HF)
                        ACT(dsA[:, sh], dsA[:, sh], act.Sqrt)
                        ACT(dsA[:, sh], dsA[:, sh], act.Sigmoid,
                            bias=prm[:, 0:1], scale=10.0)
                        if CFG["fgp"] == "inloop":
                            TTV(ssqB[:, sh], dsA[:, sh], ssqB[:, sh],
                                op.mult)
                            acM = ptmp.tile([P, 1], f32, tag="acM",
                                            bufs=2, name=f"acM{hh}")
                            TSP(iu6[:, sh], ssqB[:, sh], 1.0, None,
                                op.mult, op.add, accum_out=acM)
                            mt_done.append(acM)

                # ---- tail ----
                accB = pers.tile([P, 1], f32, tag="accB")
                accMT = pers.tile([P, 1], f32, tag="accMT")
                if CFG["fgp"] == "fpp":
                    for tt in range(NTI):
                        do_fgp(tt)
                if CFG["tail"] == "fpp":
                    HF = FPP // 2
                    for hh in range(2):
                        sh = slice(hh * HF, (hh + 1) * HF)
                        ACT(dsA[:, sh], dsA[:, sh], act.Sqrt)
                        ACT(dsA[:, sh], dsA[:, sh], act.Sigmoid,
                            bias=prm[:, 0:1], scale=10.0)
                        TTV(ssqB[:, sh], dsA[:, sh], ssqB[:, sh],
                            op.mult)
                        acM = ptmp.tile([P, 1], f32, tag="acM", bufs=2,
                                        name=f"acM{hh}")
                        TSP(iu6[:, sh], ssqB[:, sh], 1.0, None, op.mult,
                            op.add, accum_out=acM)
                        mt_done.append(acM)
                elif CFG["fgp"] == "fpp":
                    # halves tail but fgp ran post-loop: mask-mult now
                    for hh in range(2):
                        HF = FPP // 2
                        sh = slice(hh * HF, (hh + 1) * HF)
                        TTV(ssqB[:, sh], dsA[:, sh], ssqB[:, sh],
                            op.mult)
                        acM = ptmp.tile([P, 1], f32, tag="acM", bufs=2,
                                        name=f"acM{hh}")
                        TSP(iu6[:, sh], ssqB[:, sh], 1.0, None, op.mult,
                            op.add, accum_out=acM)
                        mt_done.append(acM)
                nc.vector.tensor_copy(accB, fgp_done[0])
                for acB in fgp_done[1:]:
                    TTV(accB, accB, acB, op.add)
                nc.vector.tensor_copy(accMT, mt_done[0])
                for acM in mt_done[1:]:
                    TTV(accMT, accMT, acM, op.add)
                accO = pers.tile([P, 2], f32, tag="accO")
                nc.vector.tensor_copy(accO[:, 0:1], accB)
                nc.vector.tensor_copy(accO[:, 1:2], accMT)
                nc.sync.dma_start(out_v, accO)

    nc.compile()
    return nc


def _get_nc(full_variant: bool):
    key = (bool(full_variant), tuple(sorted(CFG.items())))
    if key not in _CACHE:
        _CACHE[key] = _build(full_variant)
    return _CACHE[key]


def _run(inputs, trace=False):
    from concourse.bass_utils import run_bass_kernel_spmd

    gt = np.ascontiguousarray(np.asarray(inputs["gt"], dtype=np.float32))
    bg = np.ascontiguousarray(np.asarray(inputs["BG_map"], dtype=np.float32))
    it = int(np.asarray(inputs["iter"]))
    full = it > 300

    if full:
        fg = np.ascontiguousarray(np.asarray(inputs["FG_map"],
                                             dtype=np.float32))
        u = np.ascontiguousarray(
            np.asarray(inputs["FG_uncertainties"], dtype=np.float32)
        ).reshape(-1)
        tp = float(np.asarray(inputs["threshold_param"]))
        thr = 1.414 * (1.0 - 1.0 / (1.0 + np.exp(-tp)))
        prm = np.zeros((P, 2), dtype=np.float32)
        prm[:, 0] = np.float32(-10.0 * thr)
        prm[:, 1] = np.float32(-LN6)
        eye = np.eye(P, dtype=np.float16)

    nc = _get_nc(full)
    in_maps = []
    for c in range(N_CORES):
        sl = slice(c * NC_RAYS, (c + 1) * NC_RAYS)
        m = {"gt_s": gt[sl], "bg_s": bg[sl]}
        if full:
            m["fg_s"] = fg[sl]
            m["u_s"] = u[sl]
            m["prm_s"] = prm
            m["eye_s"] = eye
        in_maps.append(m)

    res = run_bass_kernel_spmd(nc, in_maps, core_ids=list(range(N_CORES)),
                               trace=trace)
    parts = np.stack([r["out_s"] for r in res.results])  # [8, 128, 2]
    tot = parts.astype(np.float64).sum(axis=(0, 1))      # [2]
    if full:
        val = (tot[0] - tot[1]) / N_TOTAL
    else:
        val = tot[0] / (N_TOTAL * 3)
    return np.float32(val), res


def kernel(**inputs) -> np.ndarray:
    val, _ = _run(inputs, trace=False)
    return np.asarray(val, dtype=np.float32)


# revision 51
# speedup vs baseline: 1.0195x; 1.0048x over previous
"""Trainium2 Bass kernel for nn_BGguidedLoss (BG-guided loss function).

Strategy: pure data-parallel over 8 NeuronCores. Each core processes
N/8 = 524288 rays laid out as [128 partitions x 4096 rays/partition].

v4 design notes (all w.r.t. the TimelineSim cost model):
  - fp16 intermediates: DVE tensor_tensor runs 2x (0.52 ns/elem) and
    tensor_scalar 4x (0.26 ns/elem) on packed 2-byte SBUF operands.
  - fp32 inputs are cast+deinterleaved to planar fp16 once per tile.
  - Activation-table discipline: u is loaded/cast up front, Ln+Exp run
    before the tile loop; the loop uses only Copy/Square (present in
    every table set); Sqrt+Sigmoid run once at the end -> 5 loads.
  - The idle PE (tensor) engine computes all difference tensors
    (gt-bg, gt-fg, dh, dv) as +/-identity matmuls accumulating in
    PSUM; ACT squares read PSUM directly.
  - GPSIMD (Pool) takes the comparisons, the hue divide and the
    sector assembly (scalar_tensor_tensor).
  - Per-ray loss folded as loss_i = ssqB/3 - mask*(ssqB/3 - FGp).
    FGp and t = ssqB/3 - FGp are computed inside the loop (shifted by
    SHIFT tiles); the tail is only sqrt+sigmoid+mask-mult+accumulate.
"""

import numpy as np

N_TOTAL = 4194304
N_CORES = 8
NC_RAYS = N_TOTAL // N_CORES          # 524288 rays per core
P = 128                               # partitions
FPP = NC_RAYS // P                    # 4096 rays per partition
K = 512                               # rays per partition per tile
NIT = FPP // K                        # tile iterations
NQ = 4                                # u prefetch quarters
KQ = FPP // NQ
SHIFT = 3                             # fgp tile shift
EPS = float(2.0 ** -13)               # divide guard (fp16-normal)
LN6 = float(np.log(np.float32(6.0)))

_CACHE = {}

# tunables (see sweep): tail/fgp/u/sum/dd structure modes
CFG = {
    "tail": "fpp",       # 'fpp' | 'halves'
    "fgp": "fpp",        # 'fpp' | 'inloop'
    "u": "tile",         # 'tile' | 'qdve' | 'qact' | 'qpool'
    "sum": "split",      # 'split' | 'fused'
    "dd": "fused",       # 'split' | 'fused'
    "shift": 3,
    "ffp": "dve",
}


def _build(full_variant: bool):
    import concourse.bacc as bacc
    import concourse.mybir as mybir
    import concourse.tile as tile

    f32 = mybir.dt.float32
    f16 = mybir.dt.float16
    op = mybir.AluOpType
    act = mybir.ActivationFunctionType

    nc = bacc.Bacc("TRN2", debug=False)

    # register EPS as a const AP so activation() accepts it as a bias
    _ct = nc.alloc_sbuf_tensor("const-f32-eps", [128, 1], f32)
    nc.gpsimd.memset(_ct.ap(), EPS)
    nc.const_aps.aps[(f32, EPS)] = _ct.ap()

    gt_d = nc.dram_tensor("gt_s", [NC_RAYS, 3], f32, kind="ExternalInput")
    bg_d = nc.dram_tensor("bg_s", [NC_RAYS, 3], f32, kind="ExternalInput")
    out_d = nc.dram_tensor("out_s", [P, 2], f32, kind="ExternalOutput")
    if full_variant:
        fg_d = nc.dram_tensor("fg_s", [NC_RAYS, 3], f32, kind="ExternalInput")
        u_d = nc.dram_tensor("u_s", [NC_RAYS], f32, kind="ExternalInput")
        prm_d = nc.dram_tensor("prm_s", [P, 2], f32, kind="ExternalInput")
        eye_d = nc.dram_tensor("eye_s", [P, P], f16, kind="ExternalInput")

    gt_v = gt_d.ap().rearrange("(p f) c -> p (f c)", p=P)
    bg_v = bg_d.ap().rearrange("(p f) c -> p (f c)", p=P)
    if full_variant:
        fg_v = fg_d.ap().rearrange("(p f) c -> p (f c)", p=P)
        u_v = u_d.ap().rearrange("(p f) -> p f", p=P)
    out_v = out_d.ap()

    with tile.TileContext(nc) as tc:
        with (
            tc.tile_pool(name="pin", bufs=2) as pin,
            tc.tile_pool(name="ptmp", bufs=1) as ptmp,
            tc.tile_pool(name="pers", bufs=1) as pers,
            tc.tile_pool(name="ppsum", bufs=1, space="PSUM") as ppsum,
        ):
            TTV = nc.vector.tensor_tensor
            TTP = nc.gpsimd.tensor_tensor
            STP = nc.gpsimd.scalar_tensor_tensor
            TSP = nc.vector.tensor_scalar
            ACT = nc.scalar.activation
            MM = nc.tensor.matmul

            if not full_variant:
                accT = pers.tile([P, 1], f32, tag="accT")
                nc.vector.memset(accT, 0.0)
                for t in range(NIT):
                    sl = slice(t * 3 * K, (t + 1) * 3 * K)
                    g = pin.tile([P, 3 * K], f32, tag="g", name=f"g{t}")
                    b = pin.tile([P, 3 * K], f32, tag="b", name=f"b{t}")
                    nc.sync.dma_start(g, gt_v[:, sl])
                    nc.sync.dma_start(b, bg_v[:, sl])
                    e = ptmp.tile([P, 3 * K], f32, tag="e", bufs=2,
                                  name=f"e{t}")
                    TTV(e, g, b, op.subtract)
                    ACT(e, e, act.Square)
                    acc_t = ptmp.tile([P, 1], f32, tag="acc_t", bufs=2,
                                      name=f"acc{t}")
                    TSP(e, e, 1.0, None, op.mult, op.add, accum_out=acc_t)
                    TTV(accT, accT, acc_t, op.add)
                acc2 = pers.tile([P, 2], f32, tag="acc2")
                nc.vector.memset(acc2, 0.0)
                nc.vector.tensor_copy(acc2[:, 0:1], accT)
                nc.sync.dma_start(out_v, acc2)
            else:
                prm = pers.tile([P, 2], f32, tag="prm")
                eye = pers.tile([P, P], f16, tag="eye")
                neye = pers.tile([P, P], f16, tag="neye")
                eye6 = pers.tile([P, P], f16, tag="eye6")
                ney6 = pers.tile([P, P], f16, tag="ney6")
                nc.sync.dma_start(prm, prm_d.ap())
                nc.sync.dma_start(eye, eye_d.ap())
                TSP(neye, eye, -1.0, 0.0, op.mult, op.add)
                TSP(eye6, eye, 1.0 / 6.0, 0.0, op.mult, op.add)
                TSP(ney6, eye, -1.0 / 6.0, 0.0, op.mult, op.add)

                # persistent per-ray fp16 arrays; ssqBF[:,0,:]=B, [:,1,:]=F
                ssqBF = pers.tile([P, 2 * FPP], f16, tag="ssqBF")
                ssqBFv = ssqBF.rearrange("p (i f) -> p i f", i=2)
                ssqB = ssqBFv[:, 0]
                ssqF = ssqBFv[:, 1]
                dsA = pers.tile([P, FPP], f16, tag="dsA")
                uA = pers.tile([P, FPP], f16, tag="uA")
                iu6 = pers.tile([P, FPP], f16, tag="iu6")

                lnu = uA
                fgp_done = []
                mt_done = []

                TILES = [(0, K // 2), (K // 2, K // 2)] + \
                    [(tt * K, K) for tt in range(1, NIT)]
                NTI = len(TILES)

                def do_fgp(t):
                    """t = ssqB/3 - FGp for tile t, written into ssqB."""
                    ofs, kk = TILES[t]
                    sl1 = slice(ofs, ofs + kk)
                    fgp = ptmp.tile([P, K], f16, tag="fgp", bufs=2,
                                    name=f"fgp{t}")[:, :kk]
                    TTV(fgp, ssqF[:, sl1], iu6[:, sl1], op.mult)
                    TTV(fgp, fgp, lnu[:, sl1], op.add)
                    tBt = ptmp.tile([P, K], f16, tag="tBt", bufs=2,
                                    name=f"tBt{t}")[:, :kk]
                    acB = ptmp.tile([P, 1], f32, tag="acB", bufs=NIT + 1,
                                    name=f"acB{t}")
                    TSP(tBt, ssqB[:, sl1], 1.0 / 3.0, None, op.mult,
                        op.add, accum_out=acB)
                    TTV(ssqB[:, sl1], tBt, fgp, op.subtract)
                    fgp_done.append(acB)

                for t, (ofs, kk) in enumerate(TILES):
                    sl3 = slice(ofs * 3, (ofs + kk) * 3)
                    sl1 = slice(ofs, ofs + kk)
                    gb = pin.tile([P, 6 * K], f32, tag="gb", name=f"gb{t}")
                    ff = pin.tile([P, 3 * K], f32, tag="ff", name=f"ff{t}")
                    if CFG["u"] in ("qdve", "qact") and t < NQ:
                        # u quarters interleaved with the first tiles
                        sq = slice(t * KQ, (t + 1) * KQ)
                        uq = pin.tile([P, KQ], f32, tag="uq",
                                      name=f"uq{t}")
                        nc.sync.dma_start(uq, u_v[:, sq])
                        if CFG["u"] == "qdve":
                            nc.vector.tensor_copy(uA[:, sq], uq)
                        else:
                            ACT(uA[:, sq], uq, act.Copy)
                    elif CFG["u"] == "tile":
                        uq = pin.tile([P, K], f32, tag="uq",
                                      name=f"uq{t}")
                        nc.sync.dma_start(uq[:, :kk], u_v[:, sl1])
                        ACT(uA[:, sl1], uq[:, :kk], act.Copy)
                    elif CFG["u"] == "qpool" and t < NQ:
                        sq = slice(t * KQ, (t + 1) * KQ)
                        uq = pin.tile([P, KQ], f32, tag="uq",
                                      name=f"uq{t}")
                        nc.sync.dma_start(uq, u_v[:, sq])
                        nc.gpsimd.tensor_copy(uA[:, sq], uq)
                    nc.sync.dma_start(gb[:, :3 * kk], gt_v[:, sl3])
                    nc.sync.dma_start(gb[:, 3 * kk:6 * kk], bg_v[:, sl3])
                    nc.sync.dma_start(ff[:, :3 * kk], fg_v[:, sl3])

                    # strided fp32 channel views
                    gb32 = gb[:, :6 * kk].rearrange("p (i k c) -> p c i k", i=2, c=3)
                    gb32f = gb[:, :6 * kk].rearrange("p (i k c) -> p c (i k)", i=2, c=3)

                    # planar fp16 cast: layout [P, (c, i, k)]
                    gbP = ptmp.tile([P, 6 * K], f16, tag="gbP", bufs=2,
                                    name=f"gbP{t}")
                    gbPv = gbP[:, :6 * kk].rearrange("p (c i k) -> p c i k", i=2, c=3)
                    ACT(gbPv, gb32, act.Copy)
                    ffP = ptmp.tile([P, 3 * K], f16, tag="ffP", bufs=2,
                                    name=f"ffP{t}")
                    ffPv = ffP[:, :3 * kk].rearrange("p (c k) -> p c k", c=3)
                    ff32 = ff[:, :3 * kk].rearrange("p (k c) -> p c k", c=3)
                    if CFG.get("ffp", "dve") == "pool":
                        nc.gpsimd.tensor_copy(ffPv, ff32)
                    else:
                        ACT(ffPv, ff32, act.Copy)

                    r = gbP[:, 0:2 * kk]
                    g = gbP[:, 2 * kk:4 * kk]
                    b = gbP[:, 4 * kk:6 * kk]

                    def t2k(nm, bufs=4):
                        tl = ptmp.tile([P, 2 * K], f16, tag=nm, bufs=bufs,
                                       name=f"{nm}{t}")
                        return tl[:, :2 * kk]

                    # ---- hue chain (both images batched on [P, 2K]) ----
                    # GPSIMD ISA only supports add/sub/mult tensor ops,
                    # and DVE has no divide: reciprocal_approx_fast (fp32)
                    # replaces num/d. Pool takes off-critical mults.
                    Px = t2k("t0"); TTV(Px, g, b, op.max)
                    Py = t2k("t1"); TTV(Py, g, b, op.min)
                    c1 = t2k("c1", 2)
                    TTV(c1, g, b, op.is_lt)                      # g < b
                    M = t2k("Mt", 3); TTV(M, r, Px, op.max)
                    Qw = t2k("t2"); TTV(Qw, r, Px, op.min)
                    c2 = t2k("c2", 3)
                    TTV(c2, r, Px, op.is_lt)                     # r < Px
                    m = t2k("t3"); TTV(m, r, Py, op.min)
                    d = t2k("t4"); TTV(d, M, m, op.subtract)
                    d32 = ptmp.tile([P, 2 * K], f32, tag="d32", bufs=2,
                                    name=f"d32{t}")
                    # cast + eps-guard in one ACT pass (d+EPS > 0)
                    ACT(d32[:, :2 * kk], d, act.Identity, bias=EPS)
                    # independent ops issued BEFORE the recip so DVE's
                    # in-order queue doesn't park behind the d32 wait
                    num = t2k("t5"); TTV(num, Qw, Py, op.subtract)
                    x = t2k("t3"); TTV(x, c1, c2, op.not_equal)
                    s = t2k("t1"); TSP(s, x, -2.0, 1.0, op.mult, op.add)
                    snum = t2k("t0"); TTV(snum, s, num, op.mult)
                    rc = ptmp.tile([P, 2 * K], f32, tag="rc", bufs=2,
                                   name=f"rc{t}")
                    nc.vector.reciprocal_approx_fast(rc[:, :2 * kk], d32[:, :2 * kk])
                    sn = t2k("t2")
                    TTV(sn, snum, rc[:, :2 * kk], op.mult)
                    w = t2k("t3"); TTP(w, c1, c2, op.mult)       # c1*c2
                    zc = t2k("t5"); TTV(zc, c2, w, op.add)
                    TSP(zc, zc, 2.0, 0.0, op.mult, op.add)       # 2*zc
                    H = t2k("t4"); TTV(H, zc, sn, op.add)        # 2*zc+sn
                    t6 = t2k("t3"); TSP(t6, H, 3.0, 6.0, op.is_ge, op.mult)

                    # ---- PE: difference tensors into PSUM ----
                    # pB/pF first: they only need the casts; pd (which
                    # waits for the end of the hue chain) goes last so it
                    # does not block PE's in-order queue.
                    gtPv = gbPv[:, :, 0, :]                      # [P, 3, K]
                    bgPv = gbPv[:, :, 1, :]
                    pB = ppsum.tile([P, 3 * K], f32, tag="pB")
                    pBv = pB[:, :3 * kk].rearrange("p (c k) -> p c k", c=3)
                    pF = ppsum.tile([P, 3 * K], f32, tag="pF")
                    pFv = pF[:, :3 * kk].rearrange("p (c k) -> p c k", c=3)
                    for c in range(3):
                        MM(pBv[:, c], eye, gtPv[:, c], start=True, stop=False)
                        MM(pBv[:, c], neye, bgPv[:, c], start=False, stop=True)
                        MM(pFv[:, c], eye, gtPv[:, c], start=True, stop=False)
                        MM(pFv[:, c], neye, ffPv[:, c], start=False, stop=True)

                    # pd[:, :K] = dh*(1/6) via eye6, pd[:, K:] = dv
                    # dh = (H - t6)[:, :K] - (H - t6)[:, K:]
                    pd = ppsum.tile([P, 2 * K], f32, tag="pd")
                    ph = pd[:, :kk]
                    pv = pd[:, kk:2 * kk]
                    MM(pv, eye, M[:, :kk], start=True, stop=False)
                    MM(pv, neye, M[:, kk:], start=False, stop=True)
                    MM(ph, eye6, H[:, :kk], start=True, stop=False)
                    MM(ph, ney6, H[:, kk:], start=False, stop=False)
                    MM(ph, ney6, t6[:, :kk], start=False, stop=False)
                    MM(ph, eye6, t6[:, kk:], start=False, stop=True)

                    # ---- ACT squares (PSUM -> SBUF fp16) ----
                    if CFG["sum"] == "fused":
                        # eBF2[:, :3K] = eB^2, [:, 3K:] = eF^2
                        eBF2 = ptmp.tile([P, 6 * K], f16, tag="eBF2",
                                         bufs=2, name=f"eBF2{t}")
                        ACT(eBF2[:, :3 * K], pB, act.Square)
                        ACT(eBF2[:, 3 * K:], pF, act.Square)
                        ev = eBF2.rearrange("p (i c k) -> p c i k",
                                            i=2, c=3)
                        q2 = ptmp.tile([P, 2 * K], f16, tag="q2t",
                                       bufs=2, name=f"q2t{t}")
                        q2v = q2.rearrange("p (i k) -> p i k", i=2)
                        ssq_out = ssqBFv[:, :, sl1]
                        TTV(q2v, ev[:, 0], ev[:, 1], op.add)
                        TTV(ssq_out, q2v, ev[:, 2], op.add)
                    else:
                        eB2 = ptmp.tile([P, 3 * K], f16, tag="eB2",
                                        bufs=2, name=f"eB2{t}")
                        eF2 = ptmp.tile([P, 3 * K], f16, tag="eF2",
                                        bufs=2, name=f"eF2{t}")
                        ACT(eB2[:, :3 * kk], pB[:, :3 * kk], act.Square)
                        ACT(eF2[:, :3 * kk], pF[:, :3 * kk], act.Square)
                        q = ptmp.tile([P, K], f16, tag="qt", bufs=2,
                                      name=f"qt{t}")[:, :kk]
                        TTV(q, eB2[:, :kk], eB2[:, kk:2 * kk], op.add)
                        TTV(ssqB[:, sl1], q, eB2[:, 2 * kk:3 * kk], op.add)
                        TTP(q, eF2[:, :kk], eF2[:, kk:2 * kk], op.add)
                        TTP(ssqF[:, sl1], q, eF2[:, 2 * kk:3 * kk], op.add)

                    if CFG["dd"] == "fused":
                        dd2 = ptmp.tile([P, 2 * K], f16, tag="dd2",
                                        bufs=2, name=f"dd2{t}")
                        ACT(dd2[:, :2 * kk], pd[:, :2 * kk], act.Square)
                        TTP(dsA[:, sl1], dd2[:, :kk], dd2[:, kk:2 * kk],
                            op.add)
                    else:
                        dh2 = ptmp.tile([P, K], f16, tag="dh2", bufs=2,
                                        name=f"dh2{t}")
                        dv2 = ptmp.tile([P, K], f16, tag="dv2", bufs=2,
                                        name=f"dv2{t}")
                        ACT(dh2, pd[:, :K], act.Square)
                        ACT(dv2, pd[:, K:], act.Square)
                        TTV(dsA[:, sl1], dh2, dv2, op.add)

                    if CFG["u"] in ("qdve", "qact", "qpool") and t == NQ:
                        # all u quarters cast; one table switch each
                        ACT(lnu, uA, act.Ln)
                        ACT(iu6, lnu, act.Exp, bias=prm[:, 1:2],
                            scale=-2.0)
                    elif CFG["u"] == "tile" and t == NTI - 1:
                        ACT(lnu, uA, act.Ln)
                        ACT(iu6, lnu, act.Exp, bias=prm[:, 1:2],
                            scale=-2.0)
                    SH = CFG["shift"]
                    if CFG["fgp"] == "inloop":
                        if t >= SH:
                            do_fgp(t - SH)
                        if t == NIT - 1:
                            for tt in range(NIT - SH, NIT):
                                do_fgp(tt)
                    if CFG["tail"] == "halves" and \
                            (t == NIT - 3 or t == NIT - 1):
                        # mask + mask*t for the finished half
                        hh = 0 if t == NIT - 3 else 1
                        HF = FPP // 2
                        sh = slice(hh * HF, (hh + 1) * HF)
                        ACT(dsA[:, sh], dsA[:, sh], act.Sqrt)
                        ACT(dsA[:, sh], dsA[:, sh], act.Sigmoid,
                            bias=prm[:, 0:1], scale=10.0)
                        if CFG["fgp"] == "inloop":
                            TTV(ssqB[:, sh], dsA[:, sh], ssqB[:, sh],
                                op.mult)
                            acM = ptmp.tile([P, 1], f32, tag="acM",
                                            bufs=2, name=f"acM{hh}")
                            TSP(iu6[:, sh], ssqB[:, sh], 1.0, None,
                                op.mult, op.add, accum_out=acM)
                            mt_done.append(acM)

                # ---- tail ----
                accB = pers.tile([P, 1], f32, tag="accB")
                accMT = pers.tile([P, 1], f32, tag="accMT")
                if CFG["fgp"] == "fpp":
                    for tt in range(NTI):
                        do_fgp(tt)
                if CFG["tail"] == "fpp":
                    HF = FPP // 2
                    for hh in range(2):
                        sh = slice(hh * HF, (hh + 1) * HF)
                        ACT(dsA[:, sh], dsA[:, sh], act.Sqrt)
                        ACT(dsA[:, sh], dsA[:, sh], act.Sigmoid,
                            bias=prm[:, 0:1], scale=10.0)
                        TTV(ssqB[:, sh], dsA[:, sh], ssqB[:, sh],
                            op.mult)
                        acM = ptmp.tile([P, 1], f32, tag="acM", bufs=2,
                                        name=f"acM{hh}")
                        TSP(iu6[:, sh], ssqB[:, sh], 1.0, None, op.mult,
                            op.add, accum_out=acM)
                        mt_done.append(acM)
                elif CFG["fgp"] == "fpp":
                    # halves tail but fgp ran post-loop: mask-mult now
                    for hh in range(2):
                        HF = FPP // 2
                        sh = slice(hh * HF, (hh + 1) * HF)
                        TTV(ssqB[:, sh], dsA[:, sh], ssqB[:, sh],
                            op.mult)
                        acM = ptmp.tile([P, 1], f32, tag="acM", bufs=2,
                                        name=f"acM{hh}")
                        TSP(iu6[:, sh], ssqB[:, sh], 1.0, None, op.mult,
                            op.add, accum_out=acM)
                        mt_done.append(acM)
                nc.vector.tensor_copy(accB, fgp_done[0])
                for acB in fgp_done[1:]:
                    TTV(accB, accB, acB, op.add)
                nc.vector.tensor_copy(accMT, mt_done[0])
                for acM in mt_done[1:]:
                    TTV(accMT, accMT, acM, op.add)
                accO = pers.tile([P, 2], f32, tag="accO")
                nc.vector.tensor_copy(accO[:, 0:1], accB)
                nc.vector.tensor_copy(accO[:, 1:2], accMT)
                nc.sync.dma_start(out_v, accO)

    nc.compile()
    return nc


def _get_nc(full_variant: bool):
    key = (bool(full_variant), tuple(sorted(CFG.items())))
    if key not in _CACHE:
        _CACHE[key] = _build(full_variant)
    return _CACHE[key]


def _run(inputs, trace=False):
    from concourse.bass_utils import run_bass_kernel_spmd

    gt = np.ascontiguousarray(np.asarray(inputs["gt"], dtype=np.float32))
    bg = np.ascontiguousarray(np.asarray(inputs["BG_map"], dtype=np.float32))
    it = int(np.asarray(inputs["iter"]))
    full = it > 300

    if full:
        fg = np.ascontiguousarray(np.asarray(inputs["FG_map"],
                                             dtype=np.float32))
        u = np.ascontiguousarray(
            np.asarray(inputs["FG_uncertainties"], dtype=np.float32)
        ).reshape(-1)
        tp = float(np.asarray(inputs["threshold_param"]))
        thr = 1.414 * (1.0 - 1.0 / (1.0 + np.exp(-tp)))
        prm = np.zeros((P, 2), dtype=np.float32)
        prm[:, 0] = np.float32(-10.0 * thr)
        prm[:, 1] = np.float32(-LN6)
        eye = np.eye(P, dtype=np.float16)

    nc = _get_nc(full)
    in_maps = []
    for c in range(N_CORES):
        sl = slice(c * NC_RAYS, (c + 1) * NC_RAYS)
        m = {"gt_s": gt[sl], "bg_s": bg[sl]}
        if full:
            m["fg_s"] = fg[sl]
            m["u_s"] = u[sl]
            m["prm_s"] = prm
            m["eye_s"] = eye
        in_maps.append(m)

    res = run_bass_kernel_spmd(nc, in_maps, core_ids=list(range(N_CORES)),
                               trace=trace)
    parts = np.stack([r["out_s"] for r in res.results])  # [8, 128, 2]
    tot = parts.astype(np.float64).sum(axis=(0, 1))      # [2]
    if full:
        val = (tot[0] - tot[1]) / N_TOTAL
    else:
        val = tot[0] / (N_TOTAL * 3)
    return np.float32(val), res


def kernel(**inputs) -> np.ndarray:
    val, _ = _run(inputs, trace=False)
    return np.asarray(val, dtype=np.float32)
